# revision 20
# baseline (speedup 1.0000x reference)
"""DLRM forward (embedding_lookup) Trainium2 Bass kernel.

Data-parallel over the batch (4096/8 = 512 samples per core); every core
holds the full (bf16) embedding table stack and all MLP weights.

Per 128-sample tile:
  - one indirect-DMA gather (f32 container for bf16 pairs) in l-major order;
    bag-sum pooling as 3 contiguous DVE adds.
  - MLPs run feature-major with biases folded into the matmuls via a ones
    row (one wide activation per layer).
  - 13 PE transposes ([128s, 2 tables] -> [128f, 128s]) grouped 4 per PSUM
    tile; 8 strided copies fill the sample-major Tcat^T tile
    tf [64, s*32 + e] (32-padded so gram APs are contiguous and copy
    strides are 64B-aligned).
  - per-sample Gram matmuls Z_s = Tcat_s @ Tcat_s^T (contiguous 27-col AP)
    into packed PSUM [27, 2048] per 64-sample half.
  - Zflat: pair (i, j<i) lives at zt[32*(i%4) + j, 128*(i//4) + s]; the
    PSUM->SBUF evacuation is 4 strided cast-copies per half.  Junk
    rows/cols are zero-weighted in the top-MLP weights.
  - top MLP reads zt chunks; final Sigmoid on ACT; store [1,128] per tile.

The emission order software-pipelines three tiles so the PE stream never
has a gather-dependent instruction ahead of ready work: per iteration it
emits [next-tile bottom MLP | this-tile grams (interleaved into the ACT
gaps) | prev-tile top MLP | next-tile transposes (gather has landed by
then) | next-tile first-half grams].  Weight loads are merged into 5 DMAs
and issued after tile 0's gather so nothing delays it.
No collectives needed.
"""

import numpy as np
import ml_dtypes

B, T, L, NR, M = 4096, 26, 4, 100000, 64
E27 = T + 1                      # 27 entities (bottom output + 26 tables)
SP = 32                          # tf per-sample pitch (entities padded)
NCORES = 8
BC = B // NCORES                 # 512 samples per core
TILE = 128
NT = BC // TILE                  # 4 tiles per core

_BF = ml_dtypes.bfloat16

_prog_cache = {}

ZPAD = 896
NZCH = ZPAD // 128  # 7

# merged-weight column offsets: wb1, wb2, wt1, wt2
WCAT_OFF = {"wb1": 0, "wb2": 1024, "wt1": 1152, "wt2": 2176}
WCAT_N = 2178
# merged-bias column offsets
BOFF = {"bb0": 0, "bb1": 512, "bb2": 768, "bt0": 832, "bt1": 1344,
        "bt2": 1600}
BCAT_N = 1601


def build_program():
    import concourse.bass as bass
    import concourse.mybir as mybir
    import concourse.tile as tile
    from concourse import bacc
    from concourse.masks import make_identity
    from contextlib import ExitStack

    bf16 = mybir.dt.bfloat16
    f32 = mybir.dt.float32
    i32 = mybir.dt.int32
    Relu = mybir.ActivationFunctionType.Relu
    Sigmoid = mybir.ActivationFunctionType.Sigmoid

    nc = bacc.Bacc(
        "TRN2", target_bir_lowering=False, debug=False,
        num_devices=NCORES,
    )

    def din(name, shape, dt):
        return nc.dram_tensor(name, shape, dt, kind="ExternalInput").ap()

    # table as f32 container (bf16 pairs): the vector-indirect DMA path
    # quantizes index values through the transfer dtype — bf16 corrupts any
    # index > 256, f32 is exact below 2^24.
    table = din("table", [T * NR, M // 2], f32)
    xT = din("xT", [13, BC], bf16)
    idx = din("idx", [BC, T * L], i32)          # l-major: col = l*26 + t
    wb0 = din("wb0", [13, 512], bf16)
    wt0x = din("wt0x", [64, 512], bf16)
    wt0z = din("wt0z", [128, NZCH * 512], bf16)
    wcat = din("wcat", [128, WCAT_N], bf16)
    bcat = din("bcat", [1, BCAT_N], bf16)
    out = nc.dram_tensor("out", [NT, TILE], f32, kind="ExternalOutput").ap()

    with tile.TileContext(nc) as tc, ExitStack() as ctx:
        wpool = ctx.enter_context(tc.tile_pool(name="weights", bufs=1))
        ipool = ctx.enter_context(tc.tile_pool(name="idx", bufs=2))
        epool = ctx.enter_context(tc.tile_pool(name="emb", bufs=2))
        xpool = ctx.enter_context(tc.tile_pool(name="xin", bufs=2))
        hpool = ctx.enter_context(tc.tile_pool(name="acts", bufs=2))
        tfpool = ctx.enter_context(tc.tile_pool(name="tf", bufs=3))
        zpool = ctx.enter_context(tc.tile_pool(name="ztril", bufs=1))
        opool = ctx.enter_context(tc.tile_pool(name="outs", bufs=2))
        mmpool = ctx.enter_context(tc.tile_pool(name="mlp_psum", bufs=2, space="PSUM"))
        tppool = ctx.enter_context(tc.tile_pool(name="tp_psum", bufs=2, space="PSUM"))
        gpool = ctx.enter_context(tc.tile_pool(name="gram_psum", bufs=1, space="PSUM"))

        st = {}  # cross-stage tiles: (kind, t) -> tile

        def prologue(t):
            """input DMAs + gather for tile t (no compute engines)."""
            rows = slice(t * TILE, (t + 1) * TILE)
            it = ipool.tile([TILE, T * L], i32, tag="it")
            nc.sync.dma_start(it[:], idx[rows, :])
            es4 = ipool.tile([TILE, T * L * (M // 2)], f32, tag="es4")
            nc.gpsimd.indirect_dma_start(
                out=es4[:], out_offset=None, in_=table[:],
                in_offset=bass.IndirectOffsetOnAxis(ap=it[:], axis=0),
            )
            xt = xpool.tile([13, TILE], bf16, tag="xt")
            nc.sync.dma_start(xt[:], xT[:, rows])
            st[("es4", t)] = es4
            st[("xt", t)] = xt

        # --- constants / weights (emitted after prologue(0)) ---
        prologue(0)

        t_wb0 = wpool.tile([13, 512], bf16)
        t_wt0x = wpool.tile([64, 512], bf16)
        t_wt0z = wpool.tile([128, NZCH * 512], bf16)
        t_wcat = wpool.tile([128, WCAT_N], bf16)
        t_bcat = wpool.tile([1, BCAT_N], bf16)
        for t_, d_ in [(t_wb0, wb0), (t_wt0x, wt0x), (t_wt0z, wt0z),
                       (t_wcat, wcat), (t_bcat, bcat)]:
            nc.sync.dma_start(t_[:], d_[:])
        t_wb1 = t_wcat[:, WCAT_OFF["wb1"]:WCAT_OFF["wb1"] + 1024]
        t_wb2 = t_wcat[:, WCAT_OFF["wb2"]:WCAT_OFF["wb2"] + 128]
        t_wt1 = t_wcat[:, WCAT_OFF["wt1"]:WCAT_OFF["wt1"] + 1024]
        t_wt2 = t_wcat[:, WCAT_OFF["wt2"]:WCAT_OFF["wt2"] + 2]

        def bias(name, n):
            o = BOFF[name]
            return t_bcat[:, o:o + n]

        ident = wpool.tile([128, 128], bf16)
        make_identity(nc, ident[:])
        ones = wpool.tile([1, 128], bf16)
        nc.vector.memset(ones[:], 1.0)

        # persistent Zflat tiles (3 parities); zeroed once so pad rows stay 0
        zsets = []
        for par in range(3):
            zt_ = zpool.tile([128, ZPAD], bf16, name=f"zt{par}", tag=f"zt{par}")
            nc.vector.memset(zt_[:], 0.0)
            zsets.append(zt_)

        def l0(t):
            """bottom layer 0 (13 -> 512)."""
            xt = st.pop(("xt", t))
            ps = mmpool.tile([128, 512], f32, tag="ps")
            for ob in range(4):
                sl = slice(ob * 128, (ob + 1) * 128)
                nc.tensor.matmul(ps[:, sl], lhsT=t_wb0[:, sl], rhs=xt[:],
                                 start=True, stop=False)
                nc.tensor.matmul(ps[:, sl], lhsT=bias("bb0", 512)[:, sl],
                                 rhs=ones[:], start=False, stop=True)
            h0 = hpool.tile([128, 512], bf16, tag="h0")
            nc.scalar.activation(h0[:], ps[:], Relu)
            st[("h0", t)] = h0

        def l1(t):
            h0 = st.pop(("h0", t))
            ps = mmpool.tile([128, 256], f32, tag="ps")
            for ob in range(2):
                sl = slice(ob * 128, (ob + 1) * 128)
                for kc in range(4):
                    nc.tensor.matmul(
                        ps[:, sl],
                        lhsT=t_wb1[:, kc * 256 + ob * 128: kc * 256 + (ob + 1) * 128],
                        rhs=h0[:, kc * 128:(kc + 1) * 128],
                        start=(kc == 0), stop=False)
                nc.tensor.matmul(ps[:, sl], lhsT=bias("bb1", 256)[:, sl],
                                 rhs=ones[:], start=False, stop=True)
            h1 = hpool.tile([128, 256], bf16, tag="h1")
            nc.scalar.activation(h1[:], ps[:], Relu)
            st[("h1", t)] = h1

        def l2(t):
            h1 = st.pop(("h1", t))
            tf = tfpool.tile([64, TILE * SP], bf16, tag="tf")
            tf_e = tf[:].rearrange("p (s e) -> p s e", e=SP)
            ps = mmpool.tile([64, 128], f32, tag="ps")
            for kc in range(2):
                nc.tensor.matmul(ps[:], lhsT=t_wb2[:, kc * 64:(kc + 1) * 64],
                                 rhs=h1[:, kc * 128:(kc + 1) * 128],
                                 start=(kc == 0), stop=False)
            nc.tensor.matmul(ps[:], lhsT=bias("bb2", 64), rhs=ones[:],
                             start=False, stop=True)
            nc.scalar.activation(tf_e[:, :, 0], ps[:], Relu)
            st[("tf", t)] = tf
            return tf

        def transp(t):
            """bag-sum pooling fused into PE transposes: pt accumulates the
            four l-slot transposes of each 2-table block in PSUM, so the
            gather feeds the PE directly (no DVE adds on the chain)."""
            es4 = st.pop(("es4", t))
            es4_r = es4[:].bitcast(bf16).rearrange("p (l c) -> p l c", l=L)
            tf = st[("tf", t)]
            tf_r = tf[:].rearrange("p (s g2 two) -> p s g2 two",
                                   s=TILE, two=2)
            for grp in range(4):
                npair = 4 if grp < 3 else 1
                pt = tppool.tile([128, 512], bf16, tag="pt")
                for u in range(npair):
                    k = grp * 4 + u
                    for l in range(L):
                        nc.tensor.matmul(
                            pt[:, u * 128:(u + 1) * 128],
                            lhsT=es4_r[:, l, k * 128:(k + 1) * 128],
                            rhs=ident[:], is_transpose=True,
                            start=(l == 0), stop=(l == L - 1))
                k0 = grp * 4
                src = pt[:].rearrange("p (u s) -> p s u", s=TILE)
                # top half: even tables -> odd entities e=2k+1 (g2=k, two=1)
                dst_t = tf_r[:, :, k0:k0 + npair, 1]
                dst_b = tf_r[:, :, k0 + 1:k0 + 1 + npair, 0]
                nc.vector.tensor_copy(dst_t, src[0:64, :, 0:npair])
                if grp % 2 == 0:
                    nc.scalar.copy(dst_b, src[64:128, :, 0:npair])
                else:
                    nc.vector.tensor_copy(dst_b, src[64:128, :, 0:npair])

        gmem = {"init": False}

        def grams(t, h, lo, hi):
            """per-sample grams for samples [lo, hi) of half h."""
            tf = st[("tf", t)]
            g = gpool.tile([E27, 2048], f32, tag="g")
            if not gmem["init"]:
                nc.vector.memset(g[:], 0.0)   # junk cols stay finite
                gmem["init"] = True
            for sl in range(lo, hi):
                s = h * 64 + sl
                base = 512 * (sl // 16) + 32 * (sl % 16)
                sap = tf[:, s * SP: s * SP + E27]
                nc.tensor.matmul(g[:, base:base + E27], lhsT=sap, rhs=sap,
                                 start=True, stop=True)
            return g

        def evac(t, h, g):
            g_r = g[:].rearrange("p (q r a c) -> p q r a c", q=4, r=16, c=4)
            zt = zsets[t % 3]
            zt_r = zt[:].rearrange("P (a h2 q r) -> P a h2 q r",
                                   a=NZCH, h2=2, r=16)
            for c in range(4):
                src = g_r[:, :, :, 0:NZCH, c].rearrange("p q r a -> p a q r")
                dst = zt_r[32 * c: 32 * c + E27, :, h, :, :]
                if c % 2 == 0:
                    nc.vector.tensor_copy(dst, src)
                else:
                    nc.scalar.copy(dst, src)

        def b1(t):
            """top-MLP layer 0 from zsets[t %% 3] + tf."""
            zt = zsets[t % 3]
            tf = st[("tf", t)]
            tf0 = tf[:].rearrange("p (s e) -> p s e", e=SP)[:, :, 0]
            ps = mmpool.tile([128, 512], f32, tag="ps")
            for ob in range(4):
                sl = slice(ob * 128, (ob + 1) * 128)
                nc.tensor.matmul(ps[:, sl], lhsT=t_wt0x[:, sl],
                                 rhs=tf0, start=True, stop=False)
                for a in range(NZCH):
                    nc.tensor.matmul(
                        ps[:, sl],
                        lhsT=t_wt0z[:, a * 512 + ob * 128: a * 512 + (ob + 1) * 128],
                        rhs=zt[:, a * 128:(a + 1) * 128],
                        start=False, stop=False)
                nc.tensor.matmul(ps[:, sl], lhsT=bias("bt0", 512)[:, sl],
                                 rhs=ones[:], start=False, stop=True)
            t0 = hpool.tile([128, 512], bf16, tag="t0")
            nc.scalar.activation(t0[:], ps[:], Relu)
            st[("t0", t)] = t0
            st.pop(("tf", t))

        def b2(t):
            """top-MLP layers 1..2 -> out row t."""
            t0 = st.pop(("t0", t))
            ps = mmpool.tile([128, 256], f32, tag="ps")
            for ob in range(2):
                sl = slice(ob * 128, (ob + 1) * 128)
                for kc in range(4):
                    nc.tensor.matmul(
                        ps[:, sl],
                        lhsT=t_wt1[:, kc * 256 + ob * 128: kc * 256 + (ob + 1) * 128],
                        rhs=t0[:, kc * 128:(kc + 1) * 128],
                        start=(kc == 0), stop=False)
                nc.tensor.matmul(ps[:, sl], lhsT=bias("bt1", 256)[:, sl],
                                 rhs=ones[:], start=False, stop=True)
            t1 = hpool.tile([128, 256], bf16, tag="t1")
            nc.scalar.activation(t1[:], ps[:], Relu)

            pso = mmpool.tile([1, TILE], f32, tag="ps")
            for kc in range(2):
                nc.tensor.matmul(pso[:], lhsT=t_wt2[:, kc:kc + 1],
                                 rhs=t1[:, kc * 128:(kc + 1) * 128],
                                 start=(kc == 0), stop=False)
            nc.tensor.matmul(pso[:], lhsT=bias("bt2", 1), rhs=ones[:],
                             start=False, stop=True)
            osb = opool.tile([1, TILE], f32, tag="osb")
            nc.scalar.activation(osb[:], pso[:], Sigmoid)
            nc.sync.dma_start(out[t:t + 1, :], osb[:])

        # --- software pipeline ---
        # prime tile 0
        l0(0)
        l1(0)
        l2(0)
        transp(0)
        g0 = grams(0, 0, 0, 64)
        evac(0, 0, g0)
        for t in range(NT):
            if t + 1 < NT:
                prologue(t + 1)
                l0(t + 1)
            g1 = grams(t, 1, 0, 32)
            if t + 1 < NT:
                l1(t + 1)
            grams(t, 1, 32, 64)
            evac(t, 1, g1)
            if t + 1 < NT:
                l2(t + 1)
            if t >= 1:
                b1(t - 1)
            if t + 1 < NT:
                transp(t + 1)
            if t >= 1:
                b2(t - 1)
            if t + 1 < NT:
                gn = grams(t + 1, 0, 0, 64)
                evac(t + 1, 0, gn)
        b1(NT - 1)
        b2(NT - 1)

    nc.compile()
    return nc


def _pack_k(w):
    """[K, N] with K a multiple of 128 -> [128, (K//128)*N], chunk k at
    columns [k*N, (k+1)*N)."""
    K, N = w.shape
    return np.ascontiguousarray(
        w.reshape(K // 128, 128, N).transpose(1, 0, 2).reshape(128, -1))


def _host_inputs(dense_x, sparse_idx, emb_tables,
                 bot_W0, bot_b0, bot_W1, bot_b1, bot_W2, bot_b2,
                 top_W0, top_b0, top_W1, top_b1, top_W2, top_b2):
    f32 = np.float32
    table_bf = np.ascontiguousarray(emb_tables.reshape(T * NR, M)).astype(_BF)
    table = table_bf.view(f32)                                       # [T*NR, 32]
    flat_idx = (np.asarray(sparse_idx, dtype=np.int64)
                + (np.arange(T, dtype=np.int64) * NR)[None, :, None]).astype(np.int32)
    # l-major: col = l*26 + t
    idx_tl = np.ascontiguousarray(
        flat_idx.transpose(0, 2, 1)).reshape(B, T * L)               # [B, 104]
    xTh = np.ascontiguousarray(np.asarray(dense_x, f32).T).astype(_BF)  # [13, B]

    # scatter W0z rows into the box layout: pair (i, j<i) at padded row
    # 128*(i//4) + 32*(i%4) + j
    wt0z_full = np.asarray(top_W0, f32)[:, 64:].T                     # [351, 512]
    wt0z_pad = np.zeros((ZPAD, 512), f32)
    p = 0
    for i in range(1, E27):
        r0 = 128 * (i // 4) + 32 * (i % 4)
        wt0z_pad[r0: r0 + i] = wt0z_full[p:p + i]
        p += i

    wcat_np = np.concatenate([
        _pack_k(np.asarray(bot_W1, f32).T),
        _pack_k(np.asarray(bot_W2, f32).T),
        _pack_k(np.asarray(top_W1, f32).T),
        _pack_k(np.asarray(top_W2, f32).T),
    ], axis=1)
    assert wcat_np.shape == (128, WCAT_N)
    bcat_np = np.concatenate([
        np.asarray(bot_b0, f32).reshape(1, 512),
        np.asarray(bot_b1, f32).reshape(1, 256),
        np.asarray(bot_b2, f32).reshape(1, 64),
        np.asarray(top_b0, f32).reshape(1, 512),
        np.asarray(top_b1, f32).reshape(1, 256),
        np.asarray(top_b2, f32).reshape(1, 1),
    ], axis=1)
    assert bcat_np.shape == (1, BCAT_N)

    shared = {
        "table": table,
        "wb0": np.ascontiguousarray(np.asarray(bot_W0, f32).T).astype(_BF),
        "wt0x": np.ascontiguousarray(np.asarray(top_W0, f32)[:, :64].T).astype(_BF),
        "wt0z": _pack_k(wt0z_pad).astype(_BF),
        "wcat": wcat_np.astype(_BF),
        "bcat": bcat_np.astype(_BF),
    }
    in_maps = []
    for c in range(NCORES):
        sl = slice(c * BC, (c + 1) * BC)
        m = dict(shared)
        m["xT"] = np.ascontiguousarray(xTh[:, sl])
        m["idx"] = np.ascontiguousarray(idx_tl[sl, :])
        in_maps.append(m)
    return in_maps


def kernel(**inputs):
    from concourse import bass_utils

    if "prog" not in _prog_cache:
        _prog_cache["prog"] = build_program()
    nc = _prog_cache["prog"]
    in_maps = _host_inputs(**inputs)
    res = bass_utils.run_bass_kernel_spmd(nc, in_maps, core_ids=list(range(NCORES)))
    outs = [r["out"].reshape(BC, 1) for r in res.results]
    return np.concatenate(outs, axis=0).astype(np.float32)


if __name__ == "__main__":
    prog = build_program()
    print("program built OK")


# revision 22
# speedup vs baseline: 1.0588x; 1.0588x over previous
"""DLRM forward (embedding_lookup) Trainium2 Bass kernel.

Data-parallel over the batch (4096/8 = 512 samples per core); every core
holds the full (bf16) embedding table stack and all MLP weights.

Per 128-sample tile:
  - one indirect-DMA gather (f32 container for bf16 pairs) in l-major order;
    bag-sum pooling as 3 contiguous DVE adds.
  - MLPs run feature-major with biases folded into the matmuls via a ones
    row (one wide activation per layer).
  - 13 PE transposes ([128s, 2 tables] -> [128f, 128s]) grouped 4 per PSUM
    tile; 8 strided copies fill the sample-major Tcat^T tile
    tf [64, s*32 + e] (32-padded so gram APs are contiguous and copy
    strides are 64B-aligned).
  - per-sample Gram matmuls Z_s = Tcat_s @ Tcat_s^T (contiguous 27-col AP)
    into packed PSUM [27, 2048] per 64-sample half.
  - Zflat: pair (i, j<i) lives at zt[32*(i%4) + j, 128*(i//4) + s]; the
    PSUM->SBUF evacuation is 4 strided cast-copies per half.  Junk
    rows/cols are zero-weighted in the top-MLP weights.
  - top MLP reads zt chunks; final Sigmoid on ACT; store [1,128] per tile.

The emission order software-pipelines three tiles so the PE stream never
has a gather-dependent instruction ahead of ready work: per iteration it
emits [next-tile bottom MLP | this-tile grams (interleaved into the ACT
gaps) | prev-tile top MLP | next-tile transposes (gather has landed by
then) | next-tile first-half grams].  Weight loads are merged into 5 DMAs
and issued after tile 0's gather so nothing delays it.
No collectives needed.
"""

import numpy as np
import ml_dtypes

B, T, L, NR, M = 4096, 26, 4, 100000, 64
E27 = T + 1                      # 27 entities (bottom output + 26 tables)
SP = 32                          # tf per-sample pitch (entities padded)
NCORES = 8
BC = B // NCORES                 # 512 samples per core
TILE = 128
NT = BC // TILE                  # 4 tiles per core

_BF = ml_dtypes.bfloat16

_prog_cache = {}

ZPAD = 896
NZCH = ZPAD // 128  # 7

# merged-weight column offsets: wb1, wb2, wt1, wt2
WCAT_OFF = {"wb1": 0, "wb2": 1024, "wt1": 1152, "wt2": 2176}
WCAT_N = 2178
# merged-bias column offsets
BOFF = {"bb0": 0, "bb1": 512, "bb2": 768, "bt0": 832, "bt1": 1344,
        "bt2": 1600}
BCAT_N = 1601


def build_program():
    import concourse.bass as bass
    import concourse.mybir as mybir
    import concourse.tile as tile
    from concourse import bacc
    from concourse.masks import make_identity
    from contextlib import ExitStack

    bf16 = mybir.dt.bfloat16
    f32 = mybir.dt.float32
    i32 = mybir.dt.int32
    Relu = mybir.ActivationFunctionType.Relu
    Sigmoid = mybir.ActivationFunctionType.Sigmoid

    nc = bacc.Bacc(
        "TRN2", target_bir_lowering=False, debug=False,
        num_devices=NCORES,
    )

    def din(name, shape, dt):
        return nc.dram_tensor(name, shape, dt, kind="ExternalInput").ap()

    # table as f32 container (bf16 pairs): the vector-indirect DMA path
    # quantizes index values through the transfer dtype — bf16 corrupts any
    # index > 256, f32 is exact below 2^24.
    table = din("table", [T * NR, M // 2], f32)
    xT = din("xT", [13, BC], bf16)
    idx = din("idx", [BC, T * L], i32)          # l-major: col = l*26 + t
    wb0 = din("wb0", [13, 512], bf16)
    wt0x = din("wt0x", [64, 512], bf16)
    wt0z = din("wt0z", [128, NZCH * 512], bf16)
    wcat = din("wcat", [128, WCAT_N], bf16)
    bcat = din("bcat", [1, BCAT_N], bf16)
    out = nc.dram_tensor("out", [NT, TILE], f32, kind="ExternalOutput").ap()

    with tile.TileContext(nc) as tc, ExitStack() as ctx:
        wpool = ctx.enter_context(tc.tile_pool(name="weights", bufs=1))
        ipool = ctx.enter_context(tc.tile_pool(name="idx", bufs=2))
        epool = ctx.enter_context(tc.tile_pool(name="emb", bufs=2))
        xpool = ctx.enter_context(tc.tile_pool(name="xin", bufs=2))
        hpool = ctx.enter_context(tc.tile_pool(name="acts", bufs=2))
        tfpool = ctx.enter_context(tc.tile_pool(name="tf", bufs=3))
        zpool = ctx.enter_context(tc.tile_pool(name="ztril", bufs=1))
        opool = ctx.enter_context(tc.tile_pool(name="outs", bufs=2))
        mmpool = ctx.enter_context(tc.tile_pool(name="mlp_psum", bufs=2, space="PSUM"))
        tppool = ctx.enter_context(tc.tile_pool(name="tp_psum", bufs=2, space="PSUM"))
        gpool = ctx.enter_context(tc.tile_pool(name="gram_psum", bufs=1, space="PSUM"))

        st = {}  # cross-stage tiles: (kind, t) -> tile

        def prologue(t):
            """input DMAs + gather for tile t (no compute engines)."""
            rows = slice(t * TILE, (t + 1) * TILE)
            it = ipool.tile([TILE, T * L], i32, tag="it")
            nc.sync.dma_start(it[:], idx[rows, :])
            es4 = ipool.tile([TILE, T * L * (M // 2)], f32, tag="es4")
            nc.gpsimd.indirect_dma_start(
                out=es4[:], out_offset=None, in_=table[:],
                in_offset=bass.IndirectOffsetOnAxis(ap=it[:], axis=0),
            )
            xt = xpool.tile([13, TILE], bf16, tag="xt")
            nc.sync.dma_start(xt[:], xT[:, rows])
            st[("es4", t)] = es4
            st[("xt", t)] = xt

        # --- constants / weights (emitted after prologue(0)) ---
        prologue(0)

        t_wb0 = wpool.tile([13, 512], bf16)
        t_wt0x = wpool.tile([64, 512], bf16)
        t_wt0z = wpool.tile([128, NZCH * 512], bf16)
        t_wcat = wpool.tile([128, WCAT_N], bf16)
        t_bcat = wpool.tile([1, BCAT_N], bf16)
        for t_, d_ in [(t_wb0, wb0), (t_wt0x, wt0x), (t_wt0z, wt0z),
                       (t_wcat, wcat), (t_bcat, bcat)]:
            nc.sync.dma_start(t_[:], d_[:])
        t_wb1 = t_wcat[:, WCAT_OFF["wb1"]:WCAT_OFF["wb1"] + 1024]
        t_wb2 = t_wcat[:, WCAT_OFF["wb2"]:WCAT_OFF["wb2"] + 128]
        t_wt1 = t_wcat[:, WCAT_OFF["wt1"]:WCAT_OFF["wt1"] + 1024]
        t_wt2 = t_wcat[:, WCAT_OFF["wt2"]:WCAT_OFF["wt2"] + 2]

        def bias(name, n):
            o = BOFF[name]
            return t_bcat[:, o:o + n]

        ident = wpool.tile([128, 128], bf16)
        make_identity(nc, ident[:])
        ones = wpool.tile([1, 128], bf16)
        nc.vector.memset(ones[:], 1.0)

        # persistent Zflat tiles (3 parities); zeroed once so pad rows stay 0
        zsets = []
        for par in range(3):
            zt_ = zpool.tile([128, ZPAD], bf16, name=f"zt{par}", tag=f"zt{par}")
            nc.vector.memset(zt_[:], 0.0)
            zsets.append(zt_)

        def l0(t):
            """bottom layer 0 (13 -> 512)."""
            xt = st.pop(("xt", t))
            ps = mmpool.tile([128, 512], f32, tag="ps")
            for ob in range(4):
                sl = slice(ob * 128, (ob + 1) * 128)
                nc.tensor.matmul(ps[:, sl], lhsT=t_wb0[:, sl], rhs=xt[:],
                                 start=True, stop=False)
                nc.tensor.matmul(ps[:, sl], lhsT=bias("bb0", 512)[:, sl],
                                 rhs=ones[:], start=False, stop=True)
            h0 = hpool.tile([128, 512], bf16, tag="h0")
            nc.scalar.activation(h0[:], ps[:], Relu)
            st[("h0", t)] = h0

        def l1(t):
            h0 = st.pop(("h0", t))
            ps = mmpool.tile([128, 256], f32, tag="ps")
            for ob in range(2):
                sl = slice(ob * 128, (ob + 1) * 128)
                for kc in range(4):
                    nc.tensor.matmul(
                        ps[:, sl],
                        lhsT=t_wb1[:, kc * 256 + ob * 128: kc * 256 + (ob + 1) * 128],
                        rhs=h0[:, kc * 128:(kc + 1) * 128],
                        start=(kc == 0), stop=False)
                nc.tensor.matmul(ps[:, sl], lhsT=bias("bb1", 256)[:, sl],
                                 rhs=ones[:], start=False, stop=True)
            h1 = hpool.tile([128, 256], bf16, tag="h1")
            nc.scalar.activation(h1[:], ps[:], Relu)
            st[("h1", t)] = h1

        def l2(t):
            h1 = st.pop(("h1", t))
            tf = tfpool.tile([64, TILE * (E27 + 1)], bf16, tag="tf")
            ps = mmpool.tile([64, 128], f32, tag="ps")
            for kc in range(2):
                nc.tensor.matmul(ps[:], lhsT=t_wb2[:, kc * 64:(kc + 1) * 64],
                                 rhs=h1[:, kc * 128:(kc + 1) * 128],
                                 start=(kc == 0), stop=False)
            nc.tensor.matmul(ps[:], lhsT=bias("bb2", 64), rhs=ones[:],
                             start=False, stop=True)
            nc.scalar.activation(tf[:, 0:TILE], ps[:], Relu)
            st[("tf", t)] = tf
            return tf

        def transp(t):
            """bag-sum pooling (3 DVE adds) + PE transposes into tf."""
            es4 = st.pop(("es4", t))
            es4_r = es4[:].bitcast(bf16).rearrange("p (l c) -> p l c", l=L)
            s1 = epool.tile([TILE, T * M], bf16, tag="s1")
            nc.vector.tensor_add(s1[:], es4_r[:, 0, :], es4_r[:, 1, :])
            s2 = epool.tile([TILE, T * M], bf16, tag="s2")
            nc.vector.tensor_add(s2[:], es4_r[:, 2, :], es4_r[:, 3, :])
            es = epool.tile([TILE, T * M], bf16, tag="es")
            nc.vector.tensor_add(es[:], s1[:], s2[:])
            tf = st[("tf", t)]
            tf_r = tf[:].rearrange("p (g2 two s) -> p g2 two s",
                                   two=2, s=TILE)
            for grp in range(4):
                npair = 4 if grp < 3 else 1
                pt = tppool.tile([128, 512], bf16, tag="pt")
                for u in range(npair):
                    k = grp * 4 + u
                    nc.tensor.transpose(pt[:, u * 128:(u + 1) * 128],
                                        in_=es[:, k * 128:(k + 1) * 128],
                                        identity=ident[:])
                k0 = grp * 4
                src = pt[:].rearrange("p (u s) -> p u s", s=TILE)
                # top half: even tables -> odd entities e=2k+1 (g2=k, two=1)
                dst_t = tf_r[:, k0:k0 + npair, 1, :]
                dst_b = tf_r[:, k0 + 1:k0 + 1 + npair, 0, :]
                nc.vector.tensor_copy(dst_t, src[0:64, 0:npair, :])
                if grp % 2 == 0:
                    nc.scalar.copy(dst_b, src[64:128, 0:npair, :])
                else:
                    nc.vector.tensor_copy(dst_b, src[64:128, 0:npair, :])

        gmem = {"init": False}

        def grams(t, h, lo, hi):
            """per-sample grams for samples [lo, hi) of half h."""
            tf = st[("tf", t)]
            tf_e = tf[:].rearrange("p (e s) -> p s e", s=TILE)
            g = gpool.tile([E27, 2048], f32, tag="g")
            if not gmem["init"]:
                nc.vector.memset(g[:], 0.0)   # junk cols stay finite
                gmem["init"] = True
            for sl in range(lo, hi):
                s = h * 64 + sl
                base = 512 * (sl // 16) + 32 * (sl % 16)
                sap = tf_e[:, s, 0:E27]
                nc.tensor.matmul(g[:, base:base + E27], lhsT=sap, rhs=sap,
                                 start=True, stop=True)
            return g

        def evac(t, h, g):
            g_r = g[:].rearrange("p (q r a c) -> p q r a c", q=4, r=16, c=4)
            zt = zsets[t % 3]
            zt_r = zt[:].rearrange("P (a h2 q r) -> P a h2 q r",
                                   a=NZCH, h2=2, r=16)
            for c in range(4):
                src = g_r[:, :, :, 0:NZCH, c].rearrange("p q r a -> p a q r")
                dst = zt_r[32 * c: 32 * c + E27, :, h, :, :]
                if c % 2 == 0:
                    nc.vector.tensor_copy(dst, src)
                else:
                    nc.scalar.copy(dst, src)

        def b1(t):
            """top-MLP layer 0 from zsets[t %% 3] + tf."""
            zt = zsets[t % 3]
            tf = st[("tf", t)]
            tf0 = tf[:, 0:TILE]
            ps = mmpool.tile([128, 512], f32, tag="ps")
            for ob in range(4):
                sl = slice(ob * 128, (ob + 1) * 128)
                nc.tensor.matmul(ps[:, sl], lhsT=t_wt0x[:, sl],
                                 rhs=tf0, start=True, stop=False)
                for a in range(NZCH):
                    nc.tensor.matmul(
                        ps[:, sl],
                        lhsT=t_wt0z[:, a * 512 + ob * 128: a * 512 + (ob + 1) * 128],
                        rhs=zt[:, a * 128:(a + 1) * 128],
                        start=False, stop=False)
                nc.tensor.matmul(ps[:, sl], lhsT=bias("bt0", 512)[:, sl],
                                 rhs=ones[:], start=False, stop=True)
            t0 = hpool.tile([128, 512], bf16, tag="t0")
            nc.scalar.activation(t0[:], ps[:], Relu)
            st[("t0", t)] = t0
            st.pop(("tf", t))

        def b2(t):
            """top-MLP layers 1..2 -> out row t."""
            t0 = st.pop(("t0", t))
            ps = mmpool.tile([128, 256], f32, tag="ps")
            for ob in range(2):
                sl = slice(ob * 128, (ob + 1) * 128)
                for kc in range(4):
                    nc.tensor.matmul(
                        ps[:, sl],
                        lhsT=t_wt1[:, kc * 256 + ob * 128: kc * 256 + (ob + 1) * 128],
                        rhs=t0[:, kc * 128:(kc + 1) * 128],
                        start=(kc == 0), stop=False)
                nc.tensor.matmul(ps[:, sl], lhsT=bias("bt1", 256)[:, sl],
                                 rhs=ones[:], start=False, stop=True)
            t1 = hpool.tile([128, 256], bf16, tag="t1")
            nc.scalar.activation(t1[:], ps[:], Relu)

            pso = mmpool.tile([1, TILE], f32, tag="ps")
            for kc in range(2):
                nc.tensor.matmul(pso[:], lhsT=t_wt2[:, kc:kc + 1],
                                 rhs=t1[:, kc * 128:(kc + 1) * 128],
                                 start=(kc == 0), stop=False)
            nc.tensor.matmul(pso[:], lhsT=bias("bt2", 1), rhs=ones[:],
                             start=False, stop=True)
            osb = opool.tile([1, TILE], f32, tag="osb")
            nc.scalar.activation(osb[:], pso[:], Sigmoid)
            nc.sync.dma_start(out[t:t + 1, :], osb[:])

        # --- software pipeline ---
        # prime tile 0
        l0(0)
        l1(0)
        l2(0)
        transp(0)
        g0 = grams(0, 0, 0, 64)
        evac(0, 0, g0)
        for t in range(NT):
            if t + 1 < NT:
                prologue(t + 1)
                l0(t + 1)
            g1 = grams(t, 1, 0, 32)
            if t + 1 < NT:
                l1(t + 1)
            grams(t, 1, 32, 64)
            evac(t, 1, g1)
            if t + 1 < NT:
                l2(t + 1)
            if t >= 1:
                b1(t - 1)
            if t + 1 < NT:
                transp(t + 1)
            if t >= 1:
                b2(t - 1)
            if t + 1 < NT:
                gn = grams(t + 1, 0, 0, 64)
                evac(t + 1, 0, gn)
        b1(NT - 1)
        b2(NT - 1)

    nc.compile()
    return nc


def _pack_k(w):
    """[K, N] with K a multiple of 128 -> [128, (K//128)*N], chunk k at
    columns [k*N, (k+1)*N)."""
    K, N = w.shape
    return np.ascontiguousarray(
        w.reshape(K // 128, 128, N).transpose(1, 0, 2).reshape(128, -1))


def _host_inputs(dense_x, sparse_idx, emb_tables,
                 bot_W0, bot_b0, bot_W1, bot_b1, bot_W2, bot_b2,
                 top_W0, top_b0, top_W1, top_b1, top_W2, top_b2):
    f32 = np.float32
    table_bf = np.ascontiguousarray(emb_tables.reshape(T * NR, M)).astype(_BF)
    table = table_bf.view(f32)                                       # [T*NR, 32]
    flat_idx = (np.asarray(sparse_idx, dtype=np.int64)
                + (np.arange(T, dtype=np.int64) * NR)[None, :, None]).astype(np.int32)
    # l-major: col = l*26 + t
    idx_tl = np.ascontiguousarray(
        flat_idx.transpose(0, 2, 1)).reshape(B, T * L)               # [B, 104]
    xTh = np.ascontiguousarray(np.asarray(dense_x, f32).T).astype(_BF)  # [13, B]

    # scatter W0z rows into the box layout: pair (i, j<i) at padded row
    # 128*(i//4) + 32*(i%4) + j
    wt0z_full = np.asarray(top_W0, f32)[:, 64:].T                     # [351, 512]
    wt0z_pad = np.zeros((ZPAD, 512), f32)
    p = 0
    for i in range(1, E27):
        r0 = 128 * (i // 4) + 32 * (i % 4)
        wt0z_pad[r0: r0 + i] = wt0z_full[p:p + i]
        p += i

    wcat_np = np.concatenate([
        _pack_k(np.asarray(bot_W1, f32).T),
        _pack_k(np.asarray(bot_W2, f32).T),
        _pack_k(np.asarray(top_W1, f32).T),
        _pack_k(np.asarray(top_W2, f32).T),
    ], axis=1)
    assert wcat_np.shape == (128, WCAT_N)
    bcat_np = np.concatenate([
        np.asarray(bot_b0, f32).reshape(1, 512),
        np.asarray(bot_b1, f32).reshape(1, 256),
        np.asarray(bot_b2, f32).reshape(1, 64),
        np.asarray(top_b0, f32).reshape(1, 512),
        np.asarray(top_b1, f32).reshape(1, 256),
        np.asarray(top_b2, f32).reshape(1, 1),
    ], axis=1)
    assert bcat_np.shape == (1, BCAT_N)

    shared = {
        "table": table,
        "wb0": np.ascontiguousarray(np.asarray(bot_W0, f32).T).astype(_BF),
        "wt0x": np.ascontiguousarray(np.asarray(top_W0, f32)[:, :64].T).astype(_BF),
        "wt0z": _pack_k(wt0z_pad).astype(_BF),
        "wcat": wcat_np.astype(_BF),
        "bcat": bcat_np.astype(_BF),
    }
    in_maps = []
    for c in range(NCORES):
        sl = slice(c * BC, (c + 1) * BC)
        m = dict(shared)
        m["xT"] = np.ascontiguousarray(xTh[:, sl])
        m["idx"] = np.ascontiguousarray(idx_tl[sl, :])
        in_maps.append(m)
    return in_maps


def kernel(**inputs):
    from concourse import bass_utils

    if "prog" not in _prog_cache:
        _prog_cache["prog"] = build_program()
    nc = _prog_cache["prog"]
    in_maps = _host_inputs(**inputs)
    res = bass_utils.run_bass_kernel_spmd(nc, in_maps, core_ids=list(range(NCORES)))
    outs = [r["out"].reshape(BC, 1) for r in res.results]
    return np.concatenate(outs, axis=0).astype(np.float32)


if __name__ == "__main__":
    prog = build_program()
    print("program built OK")


# revision 25
# speedup vs baseline: 1.1673x; 1.1025x over previous
"""DLRM forward (embedding_lookup) Trainium2 Bass kernel.

Data-parallel over the batch (4096/8 = 512 samples per core); every core
holds the full (bf16) embedding table stack and all MLP weights.

Per 128-sample tile:
  - one indirect-DMA gather (f32 container for bf16 pairs) in l-major order;
    bag-sum pooling as 3 contiguous DVE adds.
  - MLPs run feature-major with biases folded into the matmuls via a ones
    row (one wide activation per layer).
  - 13 PE transposes ([128s, 2 tables] -> [128f, 128s]) grouped 4 per PSUM
    tile; 8 strided copies fill the sample-major Tcat^T tile
    tf [64, s*32 + e] (32-padded so gram APs are contiguous and copy
    strides are 64B-aligned).
  - per-sample Gram matmuls Z_s = Tcat_s @ Tcat_s^T (contiguous 27-col AP)
    into packed PSUM [27, 2048] per 64-sample half.
  - Zflat: pair (i, j<i) lives at zt[32*(i%4) + j, 128*(i//4) + s]; the
    PSUM->SBUF evacuation is 4 strided cast-copies per half.  Junk
    rows/cols are zero-weighted in the top-MLP weights.
  - top MLP reads zt chunks; final Sigmoid on ACT; store [1,128] per tile.

The emission order software-pipelines three tiles so the PE stream never
has a gather-dependent instruction ahead of ready work: per iteration it
emits [next-tile bottom MLP | this-tile grams (interleaved into the ACT
gaps) | prev-tile top MLP | next-tile transposes (gather has landed by
then) | next-tile first-half grams].  Weight loads are merged into 5 DMAs
and issued after tile 0's gather so nothing delays it.
No collectives needed.
"""

import numpy as np
import ml_dtypes

B, T, L, NR, M = 4096, 26, 4, 100000, 64
E27 = T + 1                      # 27 entities (bottom output + 26 tables)
SP = 32                          # tf per-sample pitch (entities padded)
NCORES = 8
BC = B // NCORES                 # 512 samples per core
TILE = 128
NT = BC // TILE                  # 4 tiles per core

_BF = ml_dtypes.bfloat16

_prog_cache = {}

ZPAD = 896
NZCH = ZPAD // 128  # 7

# merged-weight column offsets: wb1, wb2, wt1, wt2
WCAT_OFF = {"wb1": 0, "wb2": 1024, "wt1": 1152, "wt2": 2176}
WCAT_N = 2178
# merged-bias column offsets
BOFF = {"bb0": 0, "bb1": 512, "bb2": 768, "bt0": 832, "bt1": 1344,
        "bt2": 1600}
BCAT_N = 1601


def build_program():
    import concourse.bass as bass
    import concourse.mybir as mybir
    import concourse.tile as tile
    from concourse import bacc
    from concourse.masks import make_identity
    from contextlib import ExitStack

    bf16 = mybir.dt.bfloat16
    f32 = mybir.dt.float32
    i32 = mybir.dt.int32
    Relu = mybir.ActivationFunctionType.Relu
    Sigmoid = mybir.ActivationFunctionType.Sigmoid

    nc = bacc.Bacc(
        "TRN2", target_bir_lowering=False, debug=False,
        num_devices=NCORES,
    )

    def din(name, shape, dt):
        return nc.dram_tensor(name, shape, dt, kind="ExternalInput").ap()

    # table as f32 container (bf16 pairs): the vector-indirect DMA path
    # quantizes index values through the transfer dtype — bf16 corrupts any
    # index > 256, f32 is exact below 2^24.
    table = din("table", [T * NR, M // 2], f32)
    xT = din("xT", [13, BC], bf16)
    # [128, NT*104]: col block t holds tile t's l-major idx rows
    idx = din("idx", [TILE, NT * T * L], i32)
    wb0 = din("wb0", [13, 512], bf16)
    wt0x = din("wt0x", [64, 512], bf16)
    wt0z = din("wt0z", [128, NZCH * 512], bf16)
    wcat = din("wcat", [128, WCAT_N], bf16)
    bcat = din("bcat", [1, BCAT_N], bf16)
    out = nc.dram_tensor("out", [NT, TILE], f32, kind="ExternalOutput").ap()

    with tile.TileContext(nc) as tc, ExitStack() as ctx:
        wpool = ctx.enter_context(tc.tile_pool(name="weights", bufs=1))
        ipool = ctx.enter_context(tc.tile_pool(name="idx", bufs=4))
        epool = ctx.enter_context(tc.tile_pool(name="emb", bufs=2))
        xpool = ctx.enter_context(tc.tile_pool(name="xin", bufs=2))
        hpool = ctx.enter_context(tc.tile_pool(name="acts", bufs=2))
        tfpool = ctx.enter_context(tc.tile_pool(name="tf", bufs=3))
        zpool = ctx.enter_context(tc.tile_pool(name="ztril", bufs=1))
        opool = ctx.enter_context(tc.tile_pool(name="outs", bufs=2))
        mmpool = ctx.enter_context(tc.tile_pool(name="mlp_psum", bufs=2, space="PSUM"))
        tppool = ctx.enter_context(tc.tile_pool(name="tp_psum", bufs=2, space="PSUM"))
        gpool = ctx.enter_context(tc.tile_pool(name="gram_psum", bufs=1, space="PSUM"))

        st = {}  # cross-stage tiles: (kind, t) -> tile

        # all idx rows [128, NT*104] and dense features [13, BC] in one DMA
        # each, so gathers chain back-to-back on the queue with no per-tile
        # idx wait
        it_all = ipool.tile([TILE, NT * T * L], i32, tag="it_all", bufs=1)
        nc.sync.dma_start(it_all[:], idx[:])
        xt_all = xpool.tile([13, BC], bf16, tag="xt_all", bufs=1)
        nc.sync.dma_start(xt_all[:], xT[:])

        def prologue(t):
            """indirect gather for tile t."""
            es4 = ipool.tile([TILE, T * L * (M // 2)], f32, tag="es4")
            nc.gpsimd.indirect_dma_start(
                out=es4[:], out_offset=None, in_=table[:],
                in_offset=bass.IndirectOffsetOnAxis(
                    ap=it_all[:, t * T * L:(t + 1) * T * L], axis=0),
            )
            st[("es4", t)] = es4

        # --- constants / weights (emitted after prologue(0)) ---
        prologue(0)

        t_wb0 = wpool.tile([13, 512], bf16)
        t_wt0x = wpool.tile([64, 512], bf16)
        t_wt0z = wpool.tile([128, NZCH * 512], bf16)
        t_wcat = wpool.tile([128, WCAT_N], bf16)
        t_bcat = wpool.tile([1, BCAT_N], bf16)
        for t_, d_ in [(t_wb0, wb0), (t_wt0x, wt0x), (t_wt0z, wt0z),
                       (t_wcat, wcat), (t_bcat, bcat)]:
            nc.sync.dma_start(t_[:], d_[:])
        t_wb1 = t_wcat[:, WCAT_OFF["wb1"]:WCAT_OFF["wb1"] + 1024]
        t_wb2 = t_wcat[:, WCAT_OFF["wb2"]:WCAT_OFF["wb2"] + 128]
        t_wt1 = t_wcat[:, WCAT_OFF["wt1"]:WCAT_OFF["wt1"] + 1024]
        t_wt2 = t_wcat[:, WCAT_OFF["wt2"]:WCAT_OFF["wt2"] + 2]

        def bias(name, n):
            o = BOFF[name]
            return t_bcat[:, o:o + n]

        ident = wpool.tile([128, 128], bf16)
        make_identity(nc, ident[:])
        ones = wpool.tile([1, 128], bf16)
        nc.vector.memset(ones[:], 1.0)

        # persistent Zflat tiles (3 parities); zeroed once so pad rows stay 0
        zsets = []
        for par in range(3):
            zt_ = zpool.tile([128, ZPAD], bf16, name=f"zt{par}", tag=f"zt{par}")
            nc.vector.memset(zt_[:], 0.0)
            zsets.append(zt_)

        def l0(t):
            """bottom layer 0 (13 -> 512)."""
            xt = xt_all[:, t * TILE:(t + 1) * TILE]
            ps = mmpool.tile([128, 512], f32, tag="ps")
            for ob in range(4):
                sl = slice(ob * 128, (ob + 1) * 128)
                nc.tensor.matmul(ps[:, sl], lhsT=t_wb0[:, sl], rhs=xt,
                                 start=True, stop=False)
                nc.tensor.matmul(ps[:, sl], lhsT=bias("bb0", 512)[:, sl],
                                 rhs=ones[:], start=False, stop=True)
            h0 = hpool.tile([128, 512], bf16, tag="h0")
            nc.scalar.activation(h0[:], ps[:], Relu)
            st[("h0", t)] = h0

        def l1(t):
            h0 = st.pop(("h0", t))
            ps = mmpool.tile([128, 256], f32, tag="ps")
            for ob in range(2):
                sl = slice(ob * 128, (ob + 1) * 128)
                for kc in range(4):
                    nc.tensor.matmul(
                        ps[:, sl],
                        lhsT=t_wb1[:, kc * 256 + ob * 128: kc * 256 + (ob + 1) * 128],
                        rhs=h0[:, kc * 128:(kc + 1) * 128],
                        start=(kc == 0), stop=False)
                nc.tensor.matmul(ps[:, sl], lhsT=bias("bb1", 256)[:, sl],
                                 rhs=ones[:], start=False, stop=True)
            h1 = hpool.tile([128, 256], bf16, tag="h1")
            nc.scalar.activation(h1[:], ps[:], Relu)
            st[("h1", t)] = h1

        def l2(t):
            h1 = st.pop(("h1", t))
            tf = tfpool.tile([64, TILE * SP], bf16, tag="tf")
            tf_e = tf[:].rearrange("p (s e) -> p s e", e=SP)
            ps = mmpool.tile([64, 128], f32, tag="ps")
            for kc in range(2):
                nc.tensor.matmul(ps[:], lhsT=t_wb2[:, kc * 64:(kc + 1) * 64],
                                 rhs=h1[:, kc * 128:(kc + 1) * 128],
                                 start=(kc == 0), stop=False)
            nc.tensor.matmul(ps[:], lhsT=bias("bb2", 64), rhs=ones[:],
                             start=False, stop=True)
            nc.scalar.activation(tf_e[:, :, 0], ps[:], Relu)
            st[("tf", t)] = tf
            return tf

        def transp(t):
            """bag-sum pooling (3 DVE adds) + PE transposes into tf."""
            es4 = st.pop(("es4", t))
            es4_r = es4[:].bitcast(bf16).rearrange("p (l c) -> p l c", l=L)
            s1 = epool.tile([TILE, T * M], bf16, tag="s1")
            nc.vector.tensor_add(s1[:], es4_r[:, 0, :], es4_r[:, 1, :])
            s2 = epool.tile([TILE, T * M], bf16, tag="s2")
            nc.vector.tensor_add(s2[:], es4_r[:, 2, :], es4_r[:, 3, :])
            es = epool.tile([TILE, T * M], bf16, tag="es")
            nc.vector.tensor_add(es[:], s1[:], s2[:])
            tf = st[("tf", t)]
            tf_r = tf[:].rearrange("p (s g2 two) -> p s g2 two",
                                   s=TILE, two=2)
            for grp in range(4):
                npair = 4 if grp < 3 else 1
                pt = tppool.tile([128, 512], bf16, tag="pt")
                for u in range(npair):
                    k = grp * 4 + u
                    nc.tensor.transpose(pt[:, u * 128:(u + 1) * 128],
                                        in_=es[:, k * 128:(k + 1) * 128],
                                        identity=ident[:])
                k0 = grp * 4
                src = pt[:].rearrange("p (u s) -> p s u", s=TILE)
                # top half: even tables -> odd entities e=2k+1 (g2=k, two=1)
                dst_t = tf_r[:, :, k0:k0 + npair, 1]
                dst_b = tf_r[:, :, k0 + 1:k0 + 1 + npair, 0]
                nc.vector.tensor_copy(dst_t, src[0:64, :, 0:npair])
                nc.vector.tensor_copy(dst_b, src[64:128, :, 0:npair])

        gmem = {"init": False}

        def grams(t, h, lo, hi):
            """per-sample grams for samples [lo, hi) of half h."""
            tf = st[("tf", t)]
            g = gpool.tile([E27, 2048], f32, tag="g")
            if not gmem["init"]:
                nc.vector.memset(g[:], 0.0)   # junk cols stay finite
                gmem["init"] = True
            for sl in range(lo, hi):
                s = h * 64 + sl
                base = 512 * (sl // 16) + 32 * (sl % 16)
                sap = tf[:, s * SP: s * SP + E27]
                nc.tensor.matmul(g[:, base:base + E27], lhsT=sap, rhs=sap,
                                 start=True, stop=True)
            return g

        def evac(t, h, g):
            g_r = g[:].rearrange("p (q r a c) -> p q r a c", q=4, r=16, c=4)
            zt = zsets[t % 3]
            zt_r = zt[:].rearrange("P (a h2 q r) -> P a h2 q r",
                                   a=NZCH, h2=2, r=16)
            for c in range(4):
                src = g_r[:, :, :, 0:NZCH, c].rearrange("p q r a -> p a q r")
                dst = zt_r[32 * c: 32 * c + E27, :, h, :, :]
                nc.scalar.copy(dst, src)

        def b1(t):
            """top-MLP layer 0 from zsets[t %% 3] + tf."""
            zt = zsets[t % 3]
            tf = st[("tf", t)]
            tf0 = tf[:].rearrange("p (s e) -> p s e", e=SP)[:, :, 0]
            ps = mmpool.tile([128, 512], f32, tag="ps")
            for ob in range(4):
                sl = slice(ob * 128, (ob + 1) * 128)
                nc.tensor.matmul(ps[:, sl], lhsT=t_wt0x[:, sl],
                                 rhs=tf0, start=True, stop=False)
                for a in range(NZCH):
                    nc.tensor.matmul(
                        ps[:, sl],
                        lhsT=t_wt0z[:, a * 512 + ob * 128: a * 512 + (ob + 1) * 128],
                        rhs=zt[:, a * 128:(a + 1) * 128],
                        start=False, stop=False)
                nc.tensor.matmul(ps[:, sl], lhsT=bias("bt0", 512)[:, sl],
                                 rhs=ones[:], start=False, stop=True)
            t0 = hpool.tile([128, 512], bf16, tag="t0")
            nc.scalar.activation(t0[:], ps[:], Relu)
            st[("t0", t)] = t0
            st.pop(("tf", t))

        def b2(t):
            """top-MLP layers 1..2 -> out row t."""
            t0 = st.pop(("t0", t))
            ps = mmpool.tile([128, 256], f32, tag="ps")
            for ob in range(2):
                sl = slice(ob * 128, (ob + 1) * 128)
                for kc in range(4):
                    nc.tensor.matmul(
                        ps[:, sl],
                        lhsT=t_wt1[:, kc * 256 + ob * 128: kc * 256 + (ob + 1) * 128],
                        rhs=t0[:, kc * 128:(kc + 1) * 128],
                        start=(kc == 0), stop=False)
                nc.tensor.matmul(ps[:, sl], lhsT=bias("bt1", 256)[:, sl],
                                 rhs=ones[:], start=False, stop=True)
            t1 = hpool.tile([128, 256], bf16, tag="t1")
            nc.scalar.activation(t1[:], ps[:], Relu)

            pso = mmpool.tile([1, TILE], f32, tag="ps")
            for kc in range(2):
                nc.tensor.matmul(pso[:], lhsT=t_wt2[:, kc:kc + 1],
                                 rhs=t1[:, kc * 128:(kc + 1) * 128],
                                 start=(kc == 0), stop=False)
            nc.tensor.matmul(pso[:], lhsT=bias("bt2", 1), rhs=ones[:],
                             start=False, stop=True)
            osb = opool.tile([1, TILE], f32, tag="osb")
            nc.scalar.activation(osb[:], pso[:], Sigmoid)
            nc.sync.dma_start(out[t:t + 1, :], osb[:])

        # --- software pipeline ---
        # prime tile 0
        l0(0)
        l1(0)
        l2(0)
        transp(0)
        g0 = grams(0, 0, 0, 64)
        evac(0, 0, g0)
        for t in range(NT):
            if t + 1 < NT:
                prologue(t + 1)
                l0(t + 1)
            g1 = grams(t, 1, 0, 32)
            if t + 1 < NT:
                l1(t + 1)
            grams(t, 1, 32, 64)
            evac(t, 1, g1)
            if t + 1 < NT:
                l2(t + 1)
            if t >= 1:
                b1(t - 1)
            if t + 1 < NT:
                transp(t + 1)
            if t >= 1:
                b2(t - 1)
            if t + 1 < NT:
                gn = grams(t + 1, 0, 0, 64)
                evac(t + 1, 0, gn)
        b1(NT - 1)
        b2(NT - 1)

    nc.compile()
    return nc


def _pack_k(w):
    """[K, N] with K a multiple of 128 -> [128, (K//128)*N], chunk k at
    columns [k*N, (k+1)*N)."""
    K, N = w.shape
    return np.ascontiguousarray(
        w.reshape(K // 128, 128, N).transpose(1, 0, 2).reshape(128, -1))


def _host_inputs(dense_x, sparse_idx, emb_tables,
                 bot_W0, bot_b0, bot_W1, bot_b1, bot_W2, bot_b2,
                 top_W0, top_b0, top_W1, top_b1, top_W2, top_b2):
    f32 = np.float32
    table_bf = np.ascontiguousarray(emb_tables.reshape(T * NR, M)).astype(_BF)
    table = table_bf.view(f32)                                       # [T*NR, 32]
    flat_idx = (np.asarray(sparse_idx, dtype=np.int64)
                + (np.arange(T, dtype=np.int64) * NR)[None, :, None]).astype(np.int32)
    # l-major: col = l*26 + t
    idx_tl = np.ascontiguousarray(
        flat_idx.transpose(0, 2, 1)).reshape(B, T * L)               # [B, 104]
    # per-core tile-major: [128, NT*104]
    idx_tl = idx_tl.reshape(NCORES, NT, TILE, T * L).transpose(
        0, 2, 1, 3).reshape(NCORES, TILE, NT * T * L)
    xTh = np.ascontiguousarray(np.asarray(dense_x, f32).T).astype(_BF)  # [13, B]

    # scatter W0z rows into the box layout: pair (i, j<i) at padded row
    # 128*(i//4) + 32*(i%4) + j
    wt0z_full = np.asarray(top_W0, f32)[:, 64:].T                     # [351, 512]
    wt0z_pad = np.zeros((ZPAD, 512), f32)
    p = 0
    for i in range(1, E27):
        r0 = 128 * (i // 4) + 32 * (i % 4)
        wt0z_pad[r0: r0 + i] = wt0z_full[p:p + i]
        p += i

    wcat_np = np.concatenate([
        _pack_k(np.asarray(bot_W1, f32).T),
        _pack_k(np.asarray(bot_W2, f32).T),
        _pack_k(np.asarray(top_W1, f32).T),
        _pack_k(np.asarray(top_W2, f32).T),
    ], axis=1)
    assert wcat_np.shape == (128, WCAT_N)
    bcat_np = np.concatenate([
        np.asarray(bot_b0, f32).reshape(1, 512),
        np.asarray(bot_b1, f32).reshape(1, 256),
        np.asarray(bot_b2, f32).reshape(1, 64),
        np.asarray(top_b0, f32).reshape(1, 512),
        np.asarray(top_b1, f32).reshape(1, 256),
        np.asarray(top_b2, f32).reshape(1, 1),
    ], axis=1)
    assert bcat_np.shape == (1, BCAT_N)

    shared = {
        "table": table,
        "wb0": np.ascontiguousarray(np.asarray(bot_W0, f32).T).astype(_BF),
        "wt0x": np.ascontiguousarray(np.asarray(top_W0, f32)[:, :64].T).astype(_BF),
        "wt0z": _pack_k(wt0z_pad).astype(_BF),
        "wcat": wcat_np.astype(_BF),
        "bcat": bcat_np.astype(_BF),
    }
    in_maps = []
    for c in range(NCORES):
        sl = slice(c * BC, (c + 1) * BC)
        m = dict(shared)
        m["xT"] = np.ascontiguousarray(xTh[:, sl])
        m["idx"] = np.ascontiguousarray(idx_tl[c])
        in_maps.append(m)
    return in_maps


def kernel(**inputs):
    from concourse import bass_utils

    if "prog" not in _prog_cache:
        _prog_cache["prog"] = build_program()
    nc = _prog_cache["prog"]
    in_maps = _host_inputs(**inputs)
    res = bass_utils.run_bass_kernel_spmd(nc, in_maps, core_ids=list(range(NCORES)))
    outs = [r["out"].reshape(BC, 1) for r in res.results]
    return np.concatenate(outs, axis=0).astype(np.float32)


if __name__ == "__main__":
    prog = build_program()
    print("program built OK")


# revision 29
# speedup vs baseline: 1.1734x; 1.0052x over previous
"""DLRM forward (embedding_lookup) Trainium2 Bass kernel.

Data-parallel over the batch (4096/8 = 512 samples per core); every core
holds the full (bf16) embedding table stack and all MLP weights.

Per 128-sample tile:
  - one indirect-DMA gather (f32 container for bf16 pairs) in l-major order;
    bag-sum pooling as 3 contiguous DVE adds.
  - MLPs run feature-major with biases folded into the matmuls via a ones
    row (one wide activation per layer).
  - 13 PE transposes ([128s, 2 tables] -> [128f, 128s]) grouped 4 per PSUM
    tile; 8 strided copies fill the sample-major Tcat^T tile
    tf [64, s*32 + e] (32-padded so gram APs are contiguous and copy
    strides are 64B-aligned).
  - per-sample Gram matmuls Z_s = Tcat_s @ Tcat_s^T (contiguous 27-col AP)
    into packed PSUM [27, 2048] per 64-sample half.
  - Zflat: pair (i, j<i) lives at zt[32*(i%4) + j, 128*(i//4) + s]; the
    PSUM->SBUF evacuation is 4 strided cast-copies per half.  Junk
    rows/cols are zero-weighted in the top-MLP weights.
  - top MLP reads zt chunks; final Sigmoid on ACT; store [1,128] per tile.

The emission order software-pipelines three tiles so the PE stream never
has a gather-dependent instruction ahead of ready work: per iteration it
emits [next-tile bottom MLP | this-tile grams (interleaved into the ACT
gaps) | prev-tile top MLP | next-tile transposes (gather has landed by
then) | next-tile first-half grams].  Weight loads are merged into 5 DMAs
and issued after tile 0's gather so nothing delays it.
No collectives needed.
"""

import numpy as np
import ml_dtypes

B, T, L, NR, M = 4096, 26, 4, 100000, 64
E27 = T + 1                      # 27 entities (bottom output + 26 tables)
SP = 32                          # tf per-sample pitch (entities padded)
NCORES = 8
BC = B // NCORES                 # 512 samples per core
TILE = 128
NT = BC // TILE                  # 4 tiles per core

_BF = ml_dtypes.bfloat16

_prog_cache = {}

ZPAD = 896
NZCH = ZPAD // 128  # 7

# merged-weight column offsets: wb1, wb2, wt1, wt2
WCAT_OFF = {"wb1": 0, "wb2": 1024, "wt1": 1152, "wt2": 2176}
WCAT_N = 2178
# merged-bias column offsets
BOFF = {"bb0": 0, "bb1": 512, "bb2": 768, "bt0": 832, "bt1": 1344,
        "bt2": 1600}
BCAT_N = 1601


def build_program():
    import concourse.bass as bass
    import concourse.mybir as mybir
    import concourse.tile as tile
    from concourse import bacc
    from concourse.masks import make_identity
    from contextlib import ExitStack

    bf16 = mybir.dt.bfloat16
    f32 = mybir.dt.float32
    i32 = mybir.dt.int32
    Relu = mybir.ActivationFunctionType.Relu
    Sigmoid = mybir.ActivationFunctionType.Sigmoid

    nc = bacc.Bacc(
        "TRN2", target_bir_lowering=False, debug=False,
        num_devices=NCORES,
    )

    def din(name, shape, dt):
        return nc.dram_tensor(name, shape, dt, kind="ExternalInput").ap()

    # table as f32 container (bf16 pairs): the vector-indirect DMA path
    # quantizes index values through the transfer dtype — bf16 corrupts any
    # index > 256, f32 is exact below 2^24.
    table = din("table", [T * NR, M // 2], f32)
    xT = din("xT", [13, BC], bf16)
    # [128, NT*104]: col block t holds tile t's l-major idx rows
    idx = din("idx", [TILE, NT * T * L], i32)
    wb0 = din("wb0", [13, 512], bf16)
    wt0x = din("wt0x", [64, 512], bf16)
    wt0z = din("wt0z", [128, NZCH * 512], bf16)
    wcat = din("wcat", [128, WCAT_N], bf16)
    bcat = din("bcat", [1, BCAT_N], bf16)
    out = nc.dram_tensor("out", [NT, TILE], f32, kind="ExternalOutput").ap()

    with tile.TileContext(nc) as tc, ExitStack() as ctx:
        wpool = ctx.enter_context(tc.tile_pool(name="weights", bufs=1))
        ipool = ctx.enter_context(tc.tile_pool(name="idx", bufs=4))
        epool = ctx.enter_context(tc.tile_pool(name="emb", bufs=2))
        xpool = ctx.enter_context(tc.tile_pool(name="xin", bufs=2))
        hpool = ctx.enter_context(tc.tile_pool(name="acts", bufs=2))
        tfpool = ctx.enter_context(tc.tile_pool(name="tf", bufs=4))
        zpool = ctx.enter_context(tc.tile_pool(name="ztril", bufs=1))
        opool = ctx.enter_context(tc.tile_pool(name="outs", bufs=2))
        mmpool = ctx.enter_context(tc.tile_pool(name="mlp_psum", bufs=2, space="PSUM"))
        tppool = ctx.enter_context(tc.tile_pool(name="tp_psum", bufs=2, space="PSUM"))
        gpool = ctx.enter_context(tc.tile_pool(name="gram_psum", bufs=1, space="PSUM"))

        st = {}  # cross-stage tiles: (kind, t) -> tile

        # all idx rows [128, NT*104] and dense features [13, BC] in one DMA
        # each, so gathers chain back-to-back on the queue with no per-tile
        # idx wait
        it_all = ipool.tile([TILE, NT * T * L], i32, tag="it_all", bufs=1)
        nc.sync.dma_start(it_all[:], idx[:])
        xt_all = xpool.tile([13, BC], bf16, tag="xt_all", bufs=1)
        nc.sync.dma_start(xt_all[:], xT[:])

        def prologue(t):
            """indirect gather for tile t."""
            es4 = ipool.tile([TILE, T * L * (M // 2)], f32, tag="es4")
            nc.gpsimd.indirect_dma_start(
                out=es4[:], out_offset=None, in_=table[:],
                in_offset=bass.IndirectOffsetOnAxis(
                    ap=it_all[:, t * T * L:(t + 1) * T * L], axis=0),
            )
            st[("es4", t)] = es4

        # --- constants / weights (emitted after prologue(0)) ---
        prologue(0)

        t_wb0 = wpool.tile([13, 512], bf16)
        t_wt0x = wpool.tile([64, 512], bf16)
        t_wt0z = wpool.tile([128, NZCH * 512], bf16)
        t_wcat = wpool.tile([128, WCAT_N], bf16)
        t_bcat = wpool.tile([1, BCAT_N], bf16)
        for t_, d_ in [(t_wb0, wb0), (t_wt0x, wt0x), (t_wt0z, wt0z),
                       (t_wcat, wcat), (t_bcat, bcat)]:
            nc.sync.dma_start(t_[:], d_[:])
        t_wb1 = t_wcat[:, WCAT_OFF["wb1"]:WCAT_OFF["wb1"] + 1024]
        t_wb2 = t_wcat[:, WCAT_OFF["wb2"]:WCAT_OFF["wb2"] + 128]
        t_wt1 = t_wcat[:, WCAT_OFF["wt1"]:WCAT_OFF["wt1"] + 1024]
        t_wt2 = t_wcat[:, WCAT_OFF["wt2"]:WCAT_OFF["wt2"] + 2]

        def bias(name, n):
            o = BOFF[name]
            return t_bcat[:, o:o + n]

        ident = wpool.tile([128, 128], bf16)
        make_identity(nc, ident[:])
        ones = wpool.tile([1, 128], bf16)
        nc.vector.memset(ones[:], 1.0)

        # persistent Zflat tiles (2 pair-sets, 256 samples wide); zeroed once
        zsets = []
        for par in range(2):
            zt_ = zpool.tile([128, NZCH * 256], bf16, name=f"zt{par}",
                             tag=f"zt{par}")
            nc.vector.memset(zt_[:], 0.0)
            zsets.append(zt_)
        ones2 = wpool.tile([1, 256], bf16)
        nc.vector.memset(ones2[:], 1.0)

        def l0(t):
            """bottom layer 0 (13 -> 512)."""
            xt = xt_all[:, t * TILE:(t + 1) * TILE]
            ps = mmpool.tile([128, 512], f32, tag="ps")
            for ob in range(4):
                sl = slice(ob * 128, (ob + 1) * 128)
                nc.tensor.matmul(ps[:, sl], lhsT=t_wb0[:, sl], rhs=xt,
                                 start=True, stop=False)
                nc.tensor.matmul(ps[:, sl], lhsT=bias("bb0", 512)[:, sl],
                                 rhs=ones[:], start=False, stop=True)
            h0 = hpool.tile([128, 512], bf16, tag="h0")
            nc.scalar.activation(h0[:], ps[:], Relu)
            st[("h0", t)] = h0

        def l1(t):
            h0 = st.pop(("h0", t))
            ps = mmpool.tile([128, 256], f32, tag="ps")
            for ob in range(2):
                sl = slice(ob * 128, (ob + 1) * 128)
                for kc in range(4):
                    nc.tensor.matmul(
                        ps[:, sl],
                        lhsT=t_wb1[:, kc * 256 + ob * 128: kc * 256 + (ob + 1) * 128],
                        rhs=h0[:, kc * 128:(kc + 1) * 128],
                        start=(kc == 0), stop=False)
                nc.tensor.matmul(ps[:, sl], lhsT=bias("bb1", 256)[:, sl],
                                 rhs=ones[:], start=False, stop=True)
            h1 = hpool.tile([128, 256], bf16, tag="h1")
            nc.scalar.activation(h1[:], ps[:], Relu)
            st[("h1", t)] = h1

        def l2(t):
            h1 = st.pop(("h1", t))
            tf = tfpool.tile([64, TILE * SP], bf16, tag="tf")
            tf_e = tf[:].rearrange("p (s e) -> p s e", e=SP)
            ps = mmpool.tile([64, 128], f32, tag="ps")
            for kc in range(2):
                nc.tensor.matmul(ps[:], lhsT=t_wb2[:, kc * 64:(kc + 1) * 64],
                                 rhs=h1[:, kc * 128:(kc + 1) * 128],
                                 start=(kc == 0), stop=False)
            nc.tensor.matmul(ps[:], lhsT=bias("bb2", 64), rhs=ones[:],
                             start=False, stop=True)
            nc.scalar.activation(tf_e[:, :, 0], ps[:], Relu)
            st[("tf", t)] = tf
            return tf

        def transp(t):
            """bag-sum pooling (3 DVE adds) + PE transposes into tf."""
            es4 = st.pop(("es4", t))
            es4_r = es4[:].bitcast(bf16).rearrange("p (l c) -> p l c", l=L)
            s1 = epool.tile([TILE, T * M], bf16, tag="s1")
            nc.vector.tensor_add(s1[:], es4_r[:, 0, :], es4_r[:, 1, :])
            s2 = epool.tile([TILE, T * M], bf16, tag="s2")
            nc.vector.tensor_add(s2[:], es4_r[:, 2, :], es4_r[:, 3, :])
            es = epool.tile([TILE, T * M], bf16, tag="es")
            nc.vector.tensor_add(es[:], s1[:], s2[:])
            tf = st[("tf", t)]
            tf_r = tf[:].rearrange("p (s g2 two) -> p s g2 two",
                                   s=TILE, two=2)
            for grp in range(4):
                npair = 4 if grp < 3 else 1
                pt = tppool.tile([128, 512], bf16, tag="pt")
                for u in range(npair):
                    k = grp * 4 + u
                    nc.tensor.transpose(pt[:, u * 128:(u + 1) * 128],
                                        in_=es[:, k * 128:(k + 1) * 128],
                                        identity=ident[:])
                k0 = grp * 4
                src = pt[:].rearrange("p (u s) -> p s u", s=TILE)
                # top half: even tables -> odd entities e=2k+1 (g2=k, two=1)
                dst_t = tf_r[:, :, k0:k0 + npair, 1]
                dst_b = tf_r[:, :, k0 + 1:k0 + 1 + npair, 0]
                nc.vector.tensor_copy(dst_t, src[0:64, :, 0:npair])
                nc.vector.tensor_copy(dst_b, src[64:128, :, 0:npair])

        gmem = {"init": False}

        def grams(t, h, lo, hi):
            """per-sample grams for samples [lo, hi) of half h."""
            tf = st[("tf", t)]
            g = gpool.tile([E27, 2048], f32, tag="g")
            if not gmem["init"]:
                nc.vector.memset(g[:], 0.0)   # junk cols stay finite
                gmem["init"] = True
            for sl in range(lo, hi):
                s = h * 64 + sl
                base = 512 * (sl // 16) + 32 * (sl % 16)
                sap = tf[:, s * SP: s * SP + E27]
                nc.tensor.matmul(g[:, base:base + E27], lhsT=sap, rhs=sap,
                                 start=True, stop=True)
            return g

        def evac(t, h, g):
            g_r = g[:].rearrange("p (q r a c) -> p q r a c", q=4, r=16, c=4)
            zt = zsets[(t // 2) % 2]
            u = t % 2
            zt_r = zt[:].rearrange("P (a u h2 q r) -> P a u h2 q r",
                                   a=NZCH, u=2, h2=2, r=16)
            for c in range(4):
                src = g_r[:, :, :, 0:NZCH, c].rearrange("p q r a -> p a q r")
                dst = zt_r[32 * c: 32 * c + E27, :, u, h, :, :]
                nc.scalar.copy(dst, src)

        def b1(p):
            """top-MLP layer 0 for tile pair p (256 samples)."""
            zt = zsets[p % 2]
            tfa = st[("tf", 2 * p)]
            tfb = st[("tf", 2 * p + 1)]
            tf0a = tfa[:].rearrange("p (s e) -> p s e", e=SP)[:, :, 0]
            tf0b = tfb[:].rearrange("p (s e) -> p s e", e=SP)[:, :, 0]
            t0 = hpool.tile([128, 1024], bf16, tag="t0")
            for ob in range(4):
                wsl = slice(ob * 128, (ob + 1) * 128)
                ps = mmpool.tile([128, 256], f32, tag="ps")
                for a in range(NZCH):
                    nc.tensor.matmul(
                        ps[:],
                        lhsT=t_wt0z[:, a * 512 + ob * 128: a * 512 + (ob + 1) * 128],
                        rhs=zt[:, a * 256:(a + 1) * 256],
                        start=(a == 0), stop=False)
                nc.tensor.matmul(ps[:, 0:128], lhsT=t_wt0x[:, wsl],
                                 rhs=tf0a, start=False, stop=False)
                nc.tensor.matmul(ps[:, 128:256], lhsT=t_wt0x[:, wsl],
                                 rhs=tf0b, start=False, stop=False)
                nc.tensor.matmul(ps[:], lhsT=bias("bt0", 512)[:, wsl],
                                 rhs=ones2[:], start=False, stop=True)
                nc.scalar.activation(t0[:, ob * 256:(ob + 1) * 256], ps[:],
                                     Relu)
            st[("t0", p)] = t0
            st.pop(("tf", 2 * p))
            st.pop(("tf", 2 * p + 1))

        def b2(p):
            """top-MLP layers 1..2 -> out rows 2p, 2p+1."""
            t0 = st.pop(("t0", p))
            t1 = hpool.tile([128, 512], bf16, tag="t1")
            for ob in range(2):
                wsl = slice(ob * 128, (ob + 1) * 128)
                ps = mmpool.tile([128, 256], f32, tag="ps")
                for kc in range(4):
                    nc.tensor.matmul(
                        ps[:],
                        lhsT=t_wt1[:, kc * 256 + ob * 128: kc * 256 + (ob + 1) * 128],
                        rhs=t0[:, kc * 256:(kc + 1) * 256],
                        start=(kc == 0), stop=False)
                nc.tensor.matmul(ps[:], lhsT=bias("bt1", 256)[:, wsl],
                                 rhs=ones2[:], start=False, stop=True)
                nc.scalar.activation(t1[:, ob * 256:(ob + 1) * 256], ps[:],
                                     Relu)

            pso = mmpool.tile([1, 256], f32, tag="ps")
            for kc in range(2):
                nc.tensor.matmul(pso[:], lhsT=t_wt2[:, kc:kc + 1],
                                 rhs=t1[:, kc * 256:(kc + 1) * 256],
                                 start=(kc == 0), stop=False)
            nc.tensor.matmul(pso[:], lhsT=bias("bt2", 1), rhs=ones2[:],
                             start=False, stop=True)
            osb = opool.tile([1, 256], f32, tag="osb")
            nc.scalar.activation(osb[:], pso[:], Sigmoid)
            nc.sync.dma_start(out[2 * p:2 * p + 2, :], osb[:])

        # --- software pipeline ---
        # prime tile 0
        l0(0)
        l1(0)
        l2(0)
        transp(0)
        g0 = grams(0, 0, 0, 64)
        evac(0, 0, g0)
        for t in range(NT):
            if t + 1 < NT:
                prologue(t + 1)
                l0(t + 1)
            g1 = grams(t, 1, 0, 32)
            if t + 1 < NT:
                l1(t + 1)
            grams(t, 1, 32, 64)
            evac(t, 1, g1)
            if t + 1 < NT:
                l2(t + 1)
            if t == 2:
                b1(0)
            if t + 1 < NT:
                transp(t + 1)
            if t == 2:
                b2(0)
            if t + 1 < NT:
                gn = grams(t + 1, 0, 0, 64)
                evac(t + 1, 0, gn)
        b1(1)
        b2(1)

    nc.compile()
    return nc


def _pack_k(w):
    """[K, N] with K a multiple of 128 -> [128, (K//128)*N], chunk k at
    columns [k*N, (k+1)*N)."""
    K, N = w.shape
    return np.ascontiguousarray(
        w.reshape(K // 128, 128, N).transpose(1, 0, 2).reshape(128, -1))


def _host_inputs(dense_x, sparse_idx, emb_tables,
                 bot_W0, bot_b0, bot_W1, bot_b1, bot_W2, bot_b2,
                 top_W0, top_b0, top_W1, top_b1, top_W2, top_b2):
    f32 = np.float32
    table_bf = np.ascontiguousarray(emb_tables.reshape(T * NR, M)).astype(_BF)
    table = table_bf.view(f32)                                       # [T*NR, 32]
    flat_idx = (np.asarray(sparse_idx, dtype=np.int64)
                + (np.arange(T, dtype=np.int64) * NR)[None, :, None]).astype(np.int32)
    # l-major: col = l*26 + t
    idx_tl = np.ascontiguousarray(
        flat_idx.transpose(0, 2, 1)).reshape(B, T * L)               # [B, 104]
    # per-core tile-major: [128, NT*104]
    idx_tl = idx_tl.reshape(NCORES, NT, TILE, T * L).transpose(
        0, 2, 1, 3).reshape(NCORES, TILE, NT * T * L)
    xTh = np.ascontiguousarray(np.asarray(dense_x, f32).T).astype(_BF)  # [13, B]

    # scatter W0z rows into the box layout: pair (i, j<i) at padded row
    # 128*(i//4) + 32*(i%4) + j
    wt0z_full = np.asarray(top_W0, f32)[:, 64:].T                     # [351, 512]
    wt0z_pad = np.zeros((ZPAD, 512), f32)
    p = 0
    for i in range(1, E27):
        r0 = 128 * (i // 4) + 32 * (i % 4)
        wt0z_pad[r0: r0 + i] = wt0z_full[p:p + i]
        p += i

    wcat_np = np.concatenate([
        _pack_k(np.asarray(bot_W1, f32).T),
        _pack_k(np.asarray(bot_W2, f32).T),
        _pack_k(np.asarray(top_W1, f32).T),
        _pack_k(np.asarray(top_W2, f32).T),
    ], axis=1)
    assert wcat_np.shape == (128, WCAT_N)
    bcat_np = np.concatenate([
        np.asarray(bot_b0, f32).reshape(1, 512),
        np.asarray(bot_b1, f32).reshape(1, 256),
        np.asarray(bot_b2, f32).reshape(1, 64),
        np.asarray(top_b0, f32).reshape(1, 512),
        np.asarray(top_b1, f32).reshape(1, 256),
        np.asarray(top_b2, f32).reshape(1, 1),
    ], axis=1)
    assert bcat_np.shape == (1, BCAT_N)

    shared = {
        "table": table,
        "wb0": np.ascontiguousarray(np.asarray(bot_W0, f32).T).astype(_BF),
        "wt0x": np.ascontiguousarray(np.asarray(top_W0, f32)[:, :64].T).astype(_BF),
        "wt0z": _pack_k(wt0z_pad).astype(_BF),
        "wcat": wcat_np.astype(_BF),
        "bcat": bcat_np.astype(_BF),
    }
    in_maps = []
    for c in range(NCORES):
        sl = slice(c * BC, (c + 1) * BC)
        m = dict(shared)
        m["xT"] = np.ascontiguousarray(xTh[:, sl])
        m["idx"] = np.ascontiguousarray(idx_tl[c])
        in_maps.append(m)
    return in_maps


def kernel(**inputs):
    from concourse import bass_utils

    if "prog" not in _prog_cache:
        _prog_cache["prog"] = build_program()
    nc = _prog_cache["prog"]
    in_maps = _host_inputs(**inputs)
    res = bass_utils.run_bass_kernel_spmd(nc, in_maps, core_ids=list(range(NCORES)))
    outs = [r["out"].reshape(BC, 1) for r in res.results]
    return np.concatenate(outs, axis=0).astype(np.float32)


if __name__ == "__main__":
    prog = build_program()
    print("program built OK")
